# revision 1
# baseline (speedup 1.0000x reference)
"""Trainium2 Bass kernel for nn_CC2TBAELoss (data-parallel loss over n=20000).

Strategy: pure data parallelism over 8 NeuronCores (2500 samples each, padded
to 2560 = 20 tiles of 128 samples). Each core streams its shard once from HBM
and produces per-partition partial sums for the 5 loss terms; the host does the
final (tiny) reduction and weighting.

Perf structure (v4). The kernel is HBM-bound (17.1 MB/tile-row, ~19us/tile at
~358 GB/s), so the whole design serves one goal: the DMA queues must never
idle. Two independent DMA streams:
  - sync ring: dpi, cov, x_hat, x, mdl, obs, H, drift (~2.8 MB/tile).
  - scalar ring: the two encoder_hessian chunks (~4 MB/tile); their only
    consumer is the ACT square, so issue+consume form a private self-paced
    loop on the ACT sequencer that cannot block anything else.
Late-consumed inputs (H, obs, dr feed the qv chain at t+2) get deep buffer
rotations so their buffer-free semaphores can never head-of-line block the
sync ring. Engine roles are disjoint to minimize cross-engine hops:
  - ACT: squares (dpi, enc) + MT PSUM->SBUF copy + enc DMA issues.
  - GPSIMD: the whole tangent term (sub/mul/reduce, zero hops).
  - DVE: transposes, mse, and the WHOLE qv/npv chain as one block at t+2.
  - PE: the two per-sample matmul steps (t and t+1).

Per 128-sample tile (sample index s in [0,128); R = s//32, u = s%32):
  - mse/contractive/hessian/tangent terms: fused square(+diff)+accumulate,
    single pass per element, layout-agnostic.
  - curvature term: per-sample small matmuls on the TensorEngine, packed via
    tile_position 32x32 sub-arrays. All tiles are written by exactly one DMA
    or one compute op (keeps the race detector provably clean):
      dpiT  = DVE 32x32 block-transpose of the natural dpi tile
              -> dpiT[32R + a, 32dd + u] = dpi[s, dd, a]
      covT  = DVE 32x32 block-transpose of the natural cov tile
              -> covT[32R + b, 32a + u] = cov[s, a, b]
      step1 M'_s = cov_s @ dpi_s^T     (covT slice stationary, dpiT moving)
              -> psum_M[32R + a, 8u + dd] = M'_s[a, dd]
      step2 bbt_s = dpi_s @ M'_s       (dpiT slice stationary, M' moving),
            written to PSUM with strided columns (diagonal tile_position --
            off-diagonal col groups crash this HW/toolchain):
              psum_b[32R + j, 32k + u] = bbt_s[j, k]
    so ONE DVE 32x32 block-transpose lands bbt in NATURAL
    samples-on-partitions layout: bbt_sp[s, 32k + j] = bbt_s[j, k].
    All other per-sample tensors load naturally; qv / tangent_vector / npv
    are plain DVE broadcast-multiply + segmented reductions per partition.
  Software pipeline: step1(t) at iteration t, step2 (b_stage) at t+1,
  qv/npv chain (c_stage) at t+2.
"""

import os
import sys

import numpy as np

for _p in ("/opt/trn_rl_repo", "/root/.axon_site/_ro/trn_rl_repo"):
    if os.path.isdir(_p) and _p not in sys.path:
        sys.path.insert(0, _p)

import concourse.bacc as bacc
import concourse.bass as bass
import concourse.tile as tile
from concourse import mybir
from concourse.bass_utils import run_bass_kernel_spmd

F32 = mybir.dt.float32
N_TOTAL = 20000
D = 32
DD = 8
N_CORES = 8
P = 128
PER_CORE_PAD = 2560  # 2500 padded up to a multiple of 128

_CACHE = {}


def _build(n_per_core: int, stage: int = 99) -> bass.Bass:
    assert n_per_core % P == 0
    nt = n_per_core // P
    nc = bacc.Bacc("TRN2")

    shapes = {
        "x_hat": [n_per_core, D],
        "dpi": [n_per_core, DD, D],
        "model_projection": [n_per_core, D, D],
        "decoder_hessian": [n_per_core, D, DD, DD],
        "encoder_hessian": [n_per_core, DD, D, D],
        "x": [n_per_core, D],
        "ambient_drift": [n_per_core, D],
        "ambient_cov": [n_per_core, D, D],
        "observed_projection": [n_per_core, D, D],
    }
    ins = {
        k: nc.dram_tensor(k, shp, F32, kind="ExternalInput").ap()
        for k, shp in shapes.items()
    }
    out = nc.dram_tensor("out", [P, 8], F32, kind="ExternalOutput").ap()

    AX = mybir.AxisListType
    OP = mybir.AluOpType
    ACTF = mybir.ActivationFunctionType

    with tile.TileContext(nc) as tc:
        with (
            tc.tile_pool(name="io", bufs=3) as io,
            tc.tile_pool(name="deriv", bufs=2) as dv,
            tc.tile_pool(name="accp", bufs=1) as accp,
            tc.tile_pool(name="psum", bufs=2, space="PSUM") as psp,
        ):
            zbias = accp.tile([P, 1], F32, tag="zbias")
            nc.vector.memset(zbias, 0.0)
            acc_mse = accp.tile([P, nt], F32, tag="acc_mse")
            acc_dpi = accp.tile([P, nt], F32, tag="acc_dpi")
            acc_enc = accp.tile([P, 2 * nt], F32, tag="acc_enc")
            acc_tang = accp.tile([P, nt], F32, tag="acc_tang")
            acc_curv = accp.tile([P, nt], F32, tag="acc_curv")

            def b_stage(pv):
                dpiT_v = pv["dpiT_v"]
                MT_t = pv["MT_t"]
                # step2: bbt_s = dpi_s @ M'_s -> psum_b[32R + j, 32k + u]
                psum_b = psp.tile([P, 256], F32, tag="pb")
                nc.vector.memset(psum_b, 0.0)
                pbv = psum_b.rearrange("p (k w) -> p k w", k=8)
                for u in range(32):
                    for R in range(4):
                        nc.tensor.matmul(
                            out=pbv[32 * R : 32 * R + 8, :, u],
                            lhsT=dpiT_v[32 * R : 32 * R + 32, :, u],
                            rhs=MT_t[32 * R : 32 * R + 32, 8 * u : 8 * u + 8],
                            start=True,
                            stop=True,
                            tile_position=(32 * R, 32 * R),
                        )
                # bbt_sp[s, 32k + j] = bbt_s[j, k]  (s natural = 32R + u)
                # Split transpose + split H*bbt multiply: subtile deps let
                # each half start after only half the step2 matmuls, so DVE
                # never waits on a full PE burst. The H*bbt multiply runs on
                # DVE right after each transpose half (same engine, no hop;
                # DVE is ~2x faster per element than GPSIMD here).
                bbt_sp = dv.tile([P, 256], F32, tag="bbt_sp", bufs=3)
                H_t = pv["H_t"]
                H4 = H_t.rearrange("p (i k j) -> p i k j", i=32, k=8)
                bbt_v = bbt_sp.rearrange("p (k j) -> p k j", k=8)[:, :, 0:8]
                bbt_b = bbt_v[:, None, :, :].broadcast_to((P, 32, 8, 8))
                for hh in range(2):
                    csl = slice(128 * hh, 128 * hh + 128)
                    nc.vector.transpose(
                        out=bbt_sp[:, csl], in_=psum_b[:, csl]
                    )
                    ksl = slice(4 * hh, 4 * hh + 4)
                    nc.gpsimd.tensor_mul(
                        H4[:, :, ksl, :], H4[:, :, ksl, :], bbt_b[:, :, ksl, :]
                    )

            def c_stage(pv):
                # The whole qv/npv chain for tile t-2 as ONE uninterrupted
                # DVE block: every input (H*bbt, obs, dr) is >=1 tile old,
                # so this never stalls mid-block and never ping-pongs across
                # engines. (qv[i] = sum_q (H*bbt)[i, q], H*bbt from b_stage.)
                H_t = pv["H_t"]
                obs_t = pv["obs_t"]
                dr_t = pv["dr_t"]
                tp = pv["tcol"]
                qv_t = dv.tile([P, D], F32, tag="qv")
                nc.vector.tensor_reduce(
                    out=qv_t,
                    in_=H_t.rearrange("p (i q) -> p i q", i=32),
                    axis=AX.X,
                    op=OP.add,
                )
                tt = dv.tile([P, D], F32, tag="tt")
                nc.vector.scalar_tensor_tensor(
                    out=tt, in0=qv_t, scalar=-0.5, in1=dr_t,
                    op0=OP.mult, op1=OP.add,
                )
                obs3 = obs_t.rearrange("p (r i) -> p r i", r=32)
                t_b = tt[:, None, :].broadcast_to((P, 32, 32))
                nc.vector.tensor_mul(obs3, obs3, t_b)
                Pt_t = dv.tile([P, D], F32, tag="Pt")
                nc.vector.tensor_reduce(out=Pt_t, in_=obs3, axis=AX.X, op=OP.add)
                npv_t = dv.tile([P, D], F32, tag="npv")
                nc.vector.scalar_tensor_tensor(
                    out=npv_t, in0=Pt_t, scalar=-1.0, in1=tt,
                    op0=OP.mult, op1=OP.add,
                )
                scr2 = dv.tile([P, D], F32, tag="scr2")
                nc.vector.tensor_mul(scr2, npv_t, npv_t)
                nc.vector.tensor_reduce(
                    out=acc_curv[:, tp : tp + 1], in_=scr2, axis=AX.X, op=OP.add
                )

            ENC_LAG = 2
            ECH = 4096
            BF16 = mybir.dt.bfloat16
            enc_pending = {}

            def enc_issue(tau):
                # SWDGE cast-DMA: fp32 in HBM -> bf16 in SBUF. The hessian
                # term is ~1e-6 of the loss, so bf16 rounding is irrelevant;
                # the point is halving the enc SBUF footprint (deeper bufs
                # elsewhere). Issues ride the GpSimd sequencer (SWDGE).
                esl = slice(tau * P, (tau + 1) * P)
                esrc = ins["encoder_hessian"][esl].rearrange(
                    "s a b c -> s (a b c)"
                )
                ech_t = io.tile([P, 2 * ECH], F32, tag="enc", bufs=2)
                nc.scalar.dma_start(out=ech_t, in_=esrc)
                enc_pending[tau] = ech_t

            def enc_square(tau, ec):
                ech_t = enc_pending[tau]
                csl = slice(ec * ECH, (ec + 1) * ECH)
                nc.scalar.activation(
                    out=ech_t[:, csl],
                    in_=ech_t[:, csl],
                    func=ACTF.Square,
                    bias=zbias,
                    accum_out=acc_enc[:, 2 * tau + ec : 2 * tau + ec + 1],
                )
                if ec == 1:
                    enc_pending.pop(tau)

            def tang_stage(pv):
                # square+reduce of LAST tile's (mdl-obs): the gpsimd sub
                # finished during the previous window, so these DVE ops
                # never wait mid-FIFO (the sub used to block the next
                # tile's transposes ~7us/tile).
                mdl_t = pv["mdl_t"]
                tp = pv["tcol"]
                nc.vector.tensor_mul(mdl_t, mdl_t, mdl_t)
                nc.vector.tensor_reduce(
                    out=acc_tang[:, tp : tp + 1], in_=mdl_t, axis=AX.X,
                    op=OP.add,
                )

            def mt_copy(pv):
                # PSUM->SBUF move of LAST tile's step1 output, done at the
                # START of the next iteration: step1(t-1) finished during the
                # previous window, so these never stall ACT on the PE burst.
                # Split in 4 column ranges (subtile deps) for good measure.
                psum_M = pv.pop("psum_M")
                MT_t = dv.tile([P, 256], F32, tag="MT", bufs=3)
                for q in range(4):
                    qsl = slice(64 * q, 64 * q + 64)
                    nc.scalar.copy(out=MT_t[:, qsl], in_=psum_M[:, qsl])
                pv["MT_t"] = MT_t

            prev = None
            prev2 = None
            prevtang = None
            for t in range(nt):
                sl = slice(t * P, (t + 1) * P)

                # ---- sync-ring DMAs (PE/DVE-feeding tiles first; the sync
                # sequencer carries no compute so it free-runs ahead).
                dpi_t = io.tile([P, DD * D], F32, tag="dpi", bufs=4)
                nc.sync.dma_start(
                    out=dpi_t, in_=ins["dpi"][sl].rearrange("s dd a -> s (dd a)")
                )
                cov_t = io.tile([P, D * D], F32, tag="cov", bufs=4)
                nc.sync.dma_start(
                    out=cov_t,
                    in_=ins["ambient_cov"][sl].rearrange("s a b -> s (a b)"),
                )
                xh_t = io.tile([P, D], F32, tag="xh", bufs=4)
                nc.sync.dma_start(out=xh_t, in_=ins["x_hat"][sl])
                x_t = io.tile([P, D], F32, tag="x", bufs=4)
                nc.sync.dma_start(out=x_t, in_=ins["x"][sl])
                mdl_t = io.tile([P, D * D], F32, tag="mdl", bufs=4)
                nc.sync.dma_start(
                    out=mdl_t,
                    in_=ins["model_projection"][sl].rearrange("s i j -> s (i j)"),
                )
                obs_t = io.tile([P, D * D], F32, tag="obs", bufs=6)
                nc.sync.dma_start(
                    out=obs_t,
                    in_=ins["observed_projection"][sl].rearrange("s i j -> s (i j)"),
                )
                H_t = io.tile([P, D * DD * DD], F32, tag="H", bufs=7)
                nc.sync.dma_start(
                    out=H_t,
                    in_=ins["decoder_hessian"][sl].rearrange("s i k j -> s (i k j)"),
                )
                dr_t = io.tile([P, D], F32, tag="dr", bufs=6)
                nc.sync.dma_start(out=dr_t, in_=ins["ambient_drift"][sl])
                # enc loads ride the ACT HWDGE ring; issue lags the main loop
                # by ENC_LAG tiles so the enc stream keeps the DMA engines fed
                # while the curvature pipeline drains at the end, and the ACT
                # square lags the issue by one more tile so ACT never waits on
                # a just-issued transfer.
                if t <= 1:
                    enc_issue(t)
                elif t >= 3:
                    enc_issue(t - 1)
                # ACT order: dpi2(t) first (data long landed), one enc
                # square, THEN last tile's PSUM->SBUF copies (step1(t-1)
                # is surely done by then), then the second enc square.
                dpisq = dv.tile([P, DD * D], F32, tag="dpisq")
                nc.scalar.activation(
                    out=dpisq,
                    in_=dpi_t,
                    func=ACTF.Square,
                    bias=zbias,
                    accum_out=acc_dpi[:, t : t + 1],
                )
                if t >= 2:
                    enc_square(t - 2, 0)
                if prev is not None:
                    mt_copy(prev)
                if t >= 2:
                    enc_square(t - 2, 1)
                if prevtang is not None:
                    tang_stage(prevtang)
                    prevtang = None

                if stage < 2:
                    continue
                # curvature-pipeline transposes first: dpi/cov land first in
                # the tile's DMA stream, so DVE/PE start early.
                dpiT_t = dv.tile([P, DD * D], F32, tag="dpiT", bufs=3)
                nc.vector.transpose(out=dpiT_t, in_=dpi_t)
                # dpiT_t[32R + a, 32dd + u] = dpi[32R + u, dd, a]
                covT_t = dv.tile([P, D * D], F32, tag="covT", bufs=3)
                nc.vector.transpose(out=covT_t, in_=cov_t)
                # covT_t[32R + b, 32a + u] = cov[32R + u, a, b]
                dpiT_v = dpiT_t.rearrange("p (dd u) -> p dd u", dd=8)
                covT_v = covT_t.rearrange("p (a u) -> p a u", a=32)

                # qv/npv chain for tile t-2 (single DVE block) runs FIRST
                # among DVE work: its inputs are all >=1 window old.
                if prev2 is not None:
                    c_stage(prev2)
                    prev2 = None

                # mse: sub+mul on GPSIMD (it is nearly idle), reduce on DVE.
                diff = dv.tile([P, D], F32, tag="diff")
                nc.vector.tensor_sub(diff, xh_t, x_t)
                scr = dv.tile([P, D], F32, tag="scr")
                nc.vector.tensor_mul(scr, diff, diff)
                nc.vector.tensor_reduce(
                    out=acc_mse[:, t : t + 1], in_=scr, axis=AX.X, op=OP.add
                )

                if stage < 3:
                    continue
                # B-stage for tile t-1: PE2 + bbt PSUM->SBUF transpose.
                if prev is not None:
                    b_stage(prev)
                    prev2 = prev
                    prev = None

                # step1: M'_s = cov_s @ dpi_s^T  -> psum_M[32R + a, 8u + dd]
                # (the PSUM->SBUF copy happens at the start of iteration t+1)
                psum_M = psp.tile([P, 256], F32, tag="pm")
                for u in range(32):
                    for R in range(4):
                        nc.tensor.matmul(
                            out=psum_M[32 * R : 32 * R + 32, 8 * u : 8 * u + 8],
                            lhsT=covT_v[32 * R : 32 * R + 32, :, u],
                            rhs=dpiT_v[32 * R : 32 * R + 32, :, u],
                            start=True,
                            stop=True,
                            tile_position=(32 * R, 32 * R),
                        )
                prev = dict(
                    dpiT_v=dpiT_v, psum_M=psum_M, H_t=H_t, obs_t=obs_t,
                    dr_t=dr_t, tcol=t,
                )

                # tangent LAST in program order: the gpsimd sub waits on the
                # mdl/obs DMAs, and nothing may queue behind that wait --
                # putting it earlier head-of-line blocked downstream work
                # for ~8us/tile. sub+mul on GPSIMD, reduce on DVE.
                nc.gpsimd.tensor_sub(mdl_t, mdl_t, obs_t)
                prevtang = dict(mdl_t=mdl_t, tcol=t)

            # Epilogue: the remaining lagged enc issues/squares stream on the
            # ACT ring while the curvature pipeline drains below.
            for tau in range(nt - 1, nt):
                enc_issue(tau)
            for tau in sorted(enc_pending):
                enc_square(tau, 0)
                enc_square(tau, 1)
            if prevtang is not None:
                tang_stage(prevtang)
                prevtang = None
            if prev2 is not None:
                c_stage(prev2)
                prev2 = None
            if prev is not None:
                mt_copy(prev)
                b_stage(prev)
                c_stage(prev)
                prev = None

            # ------------- final packing -------------
            outsb = accp.tile([P, 8], F32, tag="outsb")
            nc.vector.memset(outsb, 0.0)
            if stage < 5:
                nc.vector.memset(acc_curv, 0.0)
            if stage < 2:
                nc.vector.memset(acc_dpi, 0.0)
            for j, acc in enumerate([acc_mse, acc_dpi, acc_enc, acc_tang, acc_curv]):
                nc.vector.tensor_reduce(
                    out=outsb[:, j : j + 1], in_=acc, axis=AX.X, op=OP.add
                )
            nc.sync.dma_start(out=out, in_=outsb)

    nc.finalize()
    return nc


def _get_nc(n_per_core: int) -> bass.Bass:
    if n_per_core not in _CACHE:
        _CACHE[n_per_core] = _build(n_per_core)
    return _CACHE[n_per_core]


def _make_in_maps(inputs: dict, per: int, nper: int) -> list[dict]:
    in_maps = []
    for ci in range(N_CORES):
        m = {}
        for k, arr in inputs.items():
            a = np.asarray(arr)[ci * per : (ci + 1) * per].astype(
                np.float32, copy=False
            )
            if nper > per:
                pad = np.zeros((nper - per,) + a.shape[1:], np.float32)
                a = np.concatenate([a, pad], axis=0)
            m[k] = np.ascontiguousarray(a)
        in_maps.append(m)
    return in_maps


def _combine(results, n_total: int) -> np.ndarray:
    parts = np.stack([r["out"] for r in results]).astype(np.float64)
    s = parts.sum(axis=(0, 1))
    loss = s[0] / (n_total * D) + (s[1] + s[2] + s[3]) / n_total + s[4]
    return np.array(loss, dtype=np.float32)


def run(inputs: dict, trace: bool = False):
    """Returns (loss, exec_time_ns or None). Used by kernel() and test.py."""
    n_total = np.asarray(inputs["x_hat"]).shape[0]
    per = n_total // N_CORES
    nper = ((per + P - 1) // P) * P
    nc = _get_nc(nper)
    in_maps = _make_in_maps(inputs, per, nper)
    res = run_bass_kernel_spmd(
        nc, in_maps, core_ids=list(range(N_CORES)), trace=trace
    )
    return _combine(res.results, n_total), res.exec_time_ns


def kernel(**inputs) -> np.ndarray:
    loss, _ = run(inputs)
    return loss

